# revision 41
# baseline (speedup 1.0000x reference)
"""nn_AttnDecoderCell — Trainium2 Bass kernel (8 NeuronCores, data-parallel).

kernel(**inputs) takes the FULL unsharded inputs (as produced by
setup_inputs(): x[512,1024], state[512,1024], constants[512,256,1024],
w_att[2048,1], b_att[1], w_z/u_z/b_z, w_r/u_r/b_r, w_h/u_h/b_h) and returns
the full s_t [512, 1024] float32.

Sharding: batch dim split 64 rows per core; weights replicated.

Implementation notes (all heavy tensors bf16, converted host-side):
 - GRU weights DMAed into SBUF once, issued AFTER the first few constants
   tiles so single-shot compute starts immediately; resident across For_i
   iterations.
 - C streamed as half-tiles [128(l), D] per (b, lt): 13 in-flight 256KB
   tokens instead of 6 512KB ones, covering the DMA->energy->exp->v
   round-trip latency (Little's law) so the stream stays DMA-bound.
 - Attention energy E[l] = w_c . C[b,l,:] (softmax shift-invariance drops
   the state term and b_att). Three styles round-robined to balance
   DVE/ACT/Pool (custom DVE ops run at 1x; plain TT bf16 at 2x):
     F  = DVE affine_mul_reduce (fused product+reduce), ~1.13us/slot
     PA = Pool tensor_tensor product + ACT Copy-with-accum reduce
     DA = DVE tensor_tensor product + ACT Copy-with-accum reduce
   ACT reduces are pipelined 2 slots behind their products so ACT's
   in-order queue never head-of-line blocks on a late product.
 - PSUM gotcha: a start=True matmul marks its whole 2KB zero region
   pending-zero, so every accumulation group's continuation matmuls are
   emitted immediately after their start (ch-outer/lt-inner for vT).
 - exp batched every EB groups on ACT; sumexp accumulated per block into
   a single PSUM row via ones-matmul.
 - v accumulated TRANSPOSED: per (b, d-chunk), matmul with the C tile
   [128(l),128(d)] stationary and the exp column [128(l),1] moving into a
   single PSUM bank vT [128, KW, Bc]; drained with one DVE op that folds
   in 1/sumexp.
 - GRU split early/late: bias + state@W (z,r) + x@Ux (z,r) accumulate into
   4 held-open PSUM banks interleaved with the streaming loop; the tail
   does only v@Uv for r/z, then h (bias + x@Uhx + rs@Wh + v@Uhv) reusing
   r's banks after sigmoid(r) drains them. Final combine split DVE/Pool
   per 512-col chunk, output DMA per chunk.
"""

from contextlib import ExitStack

import numpy as np

import concourse.bacc as bacc
import concourse.bass as bass
import concourse.tile as tile
import concourse.mybir as mybir
from concourse.bass_utils import run_bass_kernel_spmd
from concourse.masks import make_identity

f32 = mybir.dt.float32
bf16 = mybir.dt.bfloat16
AF = mybir.ActivationFunctionType
ALU = mybir.AluOpType

B, L, D, DIN = 512, 256, 1024, 1024
N_CORES = 8
Bc = B // N_CORES          # 64 batch rows per core
LT = L // 128              # 2 l-tiles
KW = D // 128              # 8 k-tiles for W matmuls
KU = (DIN + D) // 128      # 16 k-tiles for U matmuls
NCH = D // 512             # 2 psum chunks of 512 output cols
EB = 2                     # exp batch: groups per exp/sumexp block


def _make_pattern(counts):
    total = sum(counts.values())
    acc = {k: 0.0 for k in counts}
    out = []
    for _ in range(total):
        for k in counts:
            acc[k] += counts[k] / total
        pick = max(acc, key=lambda k: acc[k])
        acc[pick] -= 1.0
        out.append(pick)
    return out


ENERGY_PATTERN = _make_pattern({"F": 58, "PA": 42, "DA": 28})


def _build(loop_n=1, c_bufs=None, scr_bufs=4, body_reps=1):
    if c_bufs is None:
        # the For_i build reserves ~2KB/partition of SBUF for loop DMA
        # scratch, so the loop variant runs one C token short
        c_bufs = 14 if loop_n == 1 else 13
    nc = bacc.Bacc("TRN2", target_bir_lowering=False, debug=False,
                   num_devices=N_CORES)
    x_d = nc.dram_tensor("x", [Bc, DIN], bf16, kind="ExternalInput").ap()
    s_d = nc.dram_tensor("state", [Bc, D], bf16, kind="ExternalInput").ap()
    c_d = nc.dram_tensor("constants", [Bc, L, D], bf16,
                         kind="ExternalInput").ap()
    watt_d = nc.dram_tensor("w_att", [2 * D, 1], bf16,
                            kind="ExternalInput").ap()
    w_g, u_g, b_g = {}, {}, {}
    for g in "zrh":
        w_g[g] = nc.dram_tensor(f"w_{g}", [D, D], bf16,
                                kind="ExternalInput").ap()
        u_g[g] = nc.dram_tensor(f"u_{g}", [DIN + D, D], bf16,
                                kind="ExternalInput").ap()
        b_g[g] = nc.dram_tensor(f"b_{g}", [D], bf16,
                                kind="ExternalInput").ap()
    o_d = nc.dram_tensor("out", [Bc, D], f32, kind="ExternalOutput").ap()

    with tile.TileContext(nc) as tc:
      perm_es = ExitStack()
      perm = perm_es.enter_context(tc.tile_pool(name="perm", bufs=1))

      # ---- iteration-invariant setup (outside the For_i timing loop) ----
      # Small tiles first; the heavy weight DMAs are issued by
      # load_weights() from inside the first iteration's stream so the
      # constants DMA (which gates all compute) goes out first.
      ident = perm.tile([128, 128], bf16)
      make_identity(nc, ident[:])
      wc_rep = perm.tile([128, D], bf16)
      nc.sync.dma_start(
          wc_rep[:],
          bass.AP(tensor=watt_d.tensor, offset=D, ap=[[0, 128], [1, D]]))
      ones_col = perm.tile([128, 1], bf16)
      nc.vector.memset(ones_col[:], 1.0)
      ones_row = perm.tile([1, Bc], bf16)
      nc.vector.memset(ones_row[:], 1.0)
      brow = {}
      for g in "zrh":
          brow[g] = perm.tile([1, D], bf16, name=f"brow_{g}")
          nc.sync.dma_start(
              brow[g][:],
              bass.AP(tensor=b_g[g].tensor, offset=0, ap=[[0, 1], [1, D]]))
      wt, ut = {}, {}
      for g in "zrh":
          wt[g] = perm.tile([128, KW, D], bf16, name=f"wt_{g}")
          ut[g] = perm.tile([128, KU, D], bf16, name=f"ut_{g}")

      # Weight DMA is issued in [128, D] chunks so the single-shot build can
      # interleave them with the constants stream (C tiles gate all compute).
      # Order: chunks needed by the early GRU part first, tail-needed last.
      def weight_chunks():
          for k in range(KW):                      # early: W_z, W_r, Ux_z, Ux_r
              for g in "zr":
                  yield wt[g][:, k, :], w_g[g][k * 128:(k + 1) * 128, :]
                  yield ut[g][:, k, :], u_g[g][k * 128:(k + 1) * 128, :]
          for k in range(KW):                      # tail: Uv_z, Uv_r, Ux_h
              for g in "zr":
                  yield (ut[g][:, KW + k, :],
                         u_g[g][(KW + k) * 128:(KW + k + 1) * 128, :])
              yield ut["h"][:, k, :], u_g["h"][k * 128:(k + 1) * 128, :]
          for k in range(KW):                      # tail-only: Uv_h then W_h
              yield (ut["h"][:, KW + k, :],
                     u_g["h"][(KW + k) * 128:(KW + k + 1) * 128, :])
          for k in range(KW):
              yield wt["h"][:, k, :], w_g["h"][k * 128:(k + 1) * 128, :]

      weights_loaded = [False]

      def load_weights():
          # bulk load (used outside the For_i loop in timing builds)
          if weights_loaded[0]:
              return
          weights_loaded[0] = True
          for dst, src in weight_chunks():
              nc.sync.dma_start(dst, src)

      def body(_i):
        es = ExitStack()
        small = es.enter_context(tc.tile_pool(name="small", bufs=1))
        cpool = es.enter_context(tc.tile_pool(name="cpool", bufs=c_bufs))
        scr = es.enter_context(tc.tile_pool(name="scr", bufs=scr_bufs))
        psT = es.enter_context(tc.tile_pool(name="psT", bufs=2, space="PSUM"))
        psA = es.enter_context(tc.tile_pool(name="psA", bufs=1, space="PSUM"))
        psV = es.enter_context(tc.tile_pool(name="psV", bufs=1, space="PSUM"))
        psG = es.enter_context(tc.tile_pool(name="psG", bufs=1, space="PSUM"))

        # xs, rs_sb and ta share one rotating slot: xs is only read by the
        # early transposes, rs_sb only by the rsT transposes, ta written last.
        tmp = es.enter_context(tc.tile_pool(name="tmp", bufs=1))
        xs = scr.tile([Bc, DIN], bf16, name="xs", tag="prod")
        nc.sync.dma_start(xs[:], x_d[:])
        ss = small.tile([Bc, D], bf16)
        nc.sync.dma_start(ss[:], s_d[:])

        # C is fetched in half-tiles [128(l), D] per (b, lt): twice the
        # in-flight tokens for the same SBUF, so the DMA->energy->exp->v
        # round-trip latency is covered (Little's law).
        ct_tiles = {}

        def fetch_half(b, lt):
            ct = cpool.tile([128, D], bf16, name="ct", tag="ct")
            nc.sync.dma_start(
                ct[:],
                bass.AP(tensor=c_d.tensor, offset=(b * L + lt * 128) * D,
                        ap=[[D, 128], [1, D]]))
            ct_tiles[(b, lt)] = ct

        halves = [(b, lt) for b in range(Bc) for lt in range(LT)]
        for i in range(min(c_bufs, len(halves))):
            fetch_half(*halves[i])
        fetch_idx = [min(c_bufs, len(halves))]

        def fetch_next(n=1):
            for _ in range(n):
                if fetch_idx[0] < len(halves):
                    fetch_half(*halves[fetch_idx[0]])
                    fetch_idx[0] += 1

        # single-shot build: trickle the weight DMAs between C-group fetches
        if not weights_loaded[0]:
            weights_loaded[0] = True
            wq = iter(weight_chunks())
        else:
            wq = iter(())

        issued = [0]

        def issue_wchunks(n):
            for _ in range(n):
                if issued[0] >= 56 and fetch_idx[0] < len(halves):
                    return  # hold tail-only weights behind all C fetches
                c = next(wq, None)
                if c is None:
                    return
                issued[0] += 1
                nc.sync.dma_start(c[0], c[1])

        def transpose_to(dst3, src2d, alternate=False):
            n = dst3.shape[1]
            for ch in range(n):
                tp = psT.tile([128, Bc], bf16, name="tp", tag="tp")
                nc.tensor.transpose(tp[:], src2d[:, ch * 128:(ch + 1) * 128],
                                    ident[:Bc, :Bc])
                if alternate and ch % 2 == 1:
                    nc.scalar.activation(out=dst3[:, ch, :], in_=tp[:],
                                         func=AF.Copy)
                else:
                    nc.vector.tensor_copy(out=dst3[:, ch, :], in_=tp[:])

        sT = small.tile([128, KW, Bc], bf16)
        transpose_to(sT, ss)
        xT = small.tile([128, KW, Bc], bf16)
        transpose_to(xT, xs)

        # ---------------- GRU early-part units (interleaved below) -------
        # psG holds 4 banks: z0, z1, r0, r1 — opened with bias (start=True),
        # accumulated through the stream, closed in the tail by the v-parts.
        gp = {}
        for g in "zr":
            for chn in range(NCH):
                gp[g, chn] = psG.tile([Bc, 512], f32, name=f"gp_{g}{chn}",
                                      tag=f"gp{'01'[chn] if g == 'z' else '23'[chn]}")

        # h's x-part accumulates into the 2 psT-pool banks during the stream
        # (the transpose scratch is idle then); copied to SBUF bf16 at stream
        # end so the banks free up for the tail transposes.
        hE = {}
        h_pre = tmp.tile([Bc, D], bf16, name="h_pre", tag="tmp3")

        def early_units():
            # yields callables, each emitting a small batch of PE work
            def bias_unit():
                for g in "zr":
                    for chn in range(NCH):
                        nc.tensor.matmul(
                            gp[g, chn][:], ones_row[:],
                            brow[g][:, chn * 512:(chn + 1) * 512],
                            start=True, stop=False, skip_group_check=True)
            yield bias_unit
            for k in range(KW):
                def w_unit(k=k):
                    for g in "zr":
                        for chn in range(NCH):
                            nc.tensor.matmul(
                                gp[g, chn][:], sT[:, k, :],
                                wt[g][:, k, chn * 512:(chn + 1) * 512],
                                start=False, stop=False, skip_group_check=True)
                yield w_unit
            for k in range(KW):
                def ux_unit(k=k):
                    for g in "zr":
                        for chn in range(NCH):
                            nc.tensor.matmul(
                                gp[g, chn][:], xT[:, k, :],
                                ut[g][:, k, chn * 512:(chn + 1) * 512],
                                start=False, stop=False, skip_group_check=True)
                yield ux_unit

            def h_bias_unit():
                for chn in range(NCH):
                    hE[chn] = psT.tile([Bc, 512], f32, name=f"hE_{chn}",
                                       tag="tp")
                    nc.tensor.matmul(hE[chn][:], ones_row[:],
                                     brow["h"][:, chn * 512:(chn + 1) * 512],
                                     start=True, stop=False,
                                     skip_group_check=True)
            yield h_bias_unit
            for k in range(KW):
                def hx_unit(k=k):
                    for chn in range(NCH):
                        nc.tensor.matmul(
                            hE[chn][:], xT[:, k, :],
                            ut["h"][:, k, chn * 512:(chn + 1) * 512],
                            start=False, stop=(k == KW - 1),
                            skip_group_check=True)
                yield hx_unit

            def h_copy_unit():
                for chn in range(NCH):
                    nc.vector.tensor_copy(
                        out=h_pre[:, chn * 512:(chn + 1) * 512],
                        in_=hE[chn][:])
            yield h_copy_unit

        early_iter = iter(early_units())
        early_done = [False]

        def emit_early(n=1):
            for _ in range(n):
                u = next(early_iter, None)
                if u is None:
                    early_done[0] = True
                    return
                u()

        # ---------------- attention stream ----------------
        eT = small.tile([128, LT, Bc], f32)
        dummy = small.tile([128, 1], bf16)
        expT = small.tile([128, LT, Bc], bf16)
        vT_ps = psV.tile([128, KW, Bc], f32)
        s_ps = psA.tile([1, Bc], f32)

        n_blocks = Bc // EB
        pending = []  # deferred (prod, acol) ACT reduces, pipelined 2 slots

        def flush_pending(keep):
            while len(pending) > keep:
                prod, acol, _ = pending.pop(0)
                nc.scalar.activation(out=prod, in_=prod, func=AF.Copy,
                                     accum_out=acol)

        def blk_stage(blk):
            # exp + sumexp + v matmuls + next fetches for a finished block;
            # called 2 slots into the following block so the ACT queue never
            # head-of-line-blocks on the block's last product.
            b0 = blk * EB
            nc.scalar.activation(out=expT[:, :, b0:b0 + EB],
                                 in_=eT[:, :, b0:b0 + EB], func=AF.Exp)
            for lt in range(LT):
                nc.tensor.matmul(s_ps[:, b0:b0 + EB], ones_col[:],
                                 expT[:, lt, b0:b0 + EB],
                                 start=(lt == 0), stop=(lt == LT - 1),
                                 skip_group_check=True)
            for bi in range(EB):
                b = b0 + bi
                # ch-outer / lt-inner: a start=True matmul marks the whole
                # 2KB PSUM zero region pending-zero, so each (ch,b) group's
                # accumulating matmul must immediately follow its start —
                # no other start to this bank in between.
                cts = [ct_tiles.pop((b, lt)) for lt in range(LT)]
                for ch in range(KW):
                    for lt in range(LT):
                        nc.tensor.matmul(
                            vT_ps[:, ch, b:b + 1],
                            cts[lt][:, ch * 128:(ch + 1) * 128],
                            expT[:, lt, b:b + 1],
                            start=(lt == 0), stop=(lt == LT - 1),
                            skip_group_check=True)
                fetch_next(LT)
                issue_wchunks(2)
            if blk >= 1:
                emit_early(1)

        slot = 0
        for b in range(Bc):
            if b >= EB and b % EB == 1:
                flush_pending(len([p for p in pending
                                   if p[2] >= (b // EB) * EB * LT]))
                blk_stage(b // EB - 1)
            for lt in range(LT):
                ct = ct_tiles[(b, lt)]
                sty = ENERGY_PATTERN[slot % len(ENERGY_PATTERN)]
                acol = eT[:, lt, b:b + 1]
                if sty == "F":
                    nc.vector.affine_mul_reduce(
                        out=dummy[:].broadcast_to([128, D]),
                        accum_out=acol,
                        in0=ct[:, :], in1=wc_rep[:],
                        scale=1.0, bias=0.0)
                else:
                    prod = scr.tile([128, D], bf16, name="prod", tag="prod")
                    peng = nc.vector if sty == "DA" else nc.gpsimd
                    peng.tensor_tensor(out=prod[:], in0=ct[:, :],
                                       in1=wc_rep[:], op=ALU.mult)
                    pending.append((prod[:], acol, slot))
                slot += 1
                flush_pending(2 if b < Bc - 3 else 0)
        flush_pending(0)
        blk_stage(n_blocks - 1)
        while not early_done[0]:
            emit_early(1)
        issue_wchunks(1000)

        # ---------------- softmax normalize + v drain ----------------
        recip_row = small.tile([1, Bc], f32)
        nc.vector.reciprocal(recip_row[:], s_ps[:])
        recip_rep = small.tile([128, Bc], f32)
        nc.gpsimd.partition_broadcast(recip_rep[:], recip_row[:])

        vT_sb = small.tile([128, KW, Bc], bf16)
        nc.vector.tensor_tensor(
            out=vT_sb[:], in0=vT_ps[:],
            in1=recip_rep[:, None, :].broadcast_to([128, KW, Bc]),
            op=ALU.mult)

        # ---------------- GRU tail ----------------
        # Sigmoid is avoided (it lives in a different ACT table set than
        # exp): sigma(x) = 0.5*tanh(x/2) + 0.5, with the affine folded into
        # the DVE affine_mul_reduce consumers.
        junk = small.tile([Bc, 1], f32)

        # r v-parts first: they close r's banks earliest.
        for k in range(KW):
            for chn in range(NCH):
                nc.tensor.matmul(
                    gp["r", chn][:], vT_sb[:, k, :],
                    ut["r"][:, KW + k, chn * 512:(chn + 1) * 512],
                    start=False, stop=(k == KW - 1), skip_group_check=True)
        t_r = tmp.tile([Bc, D], bf16, name="t_r", tag="tmp2")
        for chn in range(NCH):
            nc.scalar.activation(out=t_r[:, chn * 512:(chn + 1) * 512],
                                 in_=gp["r", chn][:], func=AF.Tanh,
                                 scale=0.5)
        # z v-parts (PE) while tanh(r)/rs run on ACT/DVE
        for k in range(KW):
            for chn in range(NCH):
                nc.tensor.matmul(
                    gp["z", chn][:], vT_sb[:, k, :],
                    ut["z"][:, KW + k, chn * 512:(chn + 1) * 512],
                    start=False, stop=(k == KW - 1), skip_group_check=True)
        # rs = ss * (0.5*t_r + 0.5)
        rs_sb = tmp.tile([Bc, D], bf16, name="rs_sb", tag="tmp")
        nc.vector.affine_mul_reduce(out=rs_sb[:], accum_out=junk[:],
                                    in0=t_r[:], in1=ss[:],
                                    scale=0.5, bias=0.5)
        t_z = small.tile([Bc, D], bf16)
        for chn in range(NCH):
            nc.scalar.activation(out=t_z[:, chn * 512:(chn + 1) * 512],
                                 in_=gp["z", chn][:], func=AF.Tanh,
                                 scale=0.5)
        # zs = ss * z = ss * (0.5*t_z + 0.5)  (DVE, off critical path)
        zs_sb = small.tile([Bc, D], bf16)
        nc.vector.affine_mul_reduce(out=zs_sb[:], accum_out=junk[:],
                                    in0=t_z[:], in1=ss[:],
                                    scale=0.5, bias=0.5)

        # h gate reuses r's banks: start by re-adding the streamed x-part,
        # then Uv (no rsT dependency), then Wh(rsT) last.
        hp = {}
        for chn in range(NCH):
            hp[chn] = psG.tile([Bc, 512], f32, name=f"hp_{chn}",
                               tag=f"gp{'23'[chn]}")
            nc.tensor.matmul(hp[chn][:], ident[:Bc, :Bc],
                             h_pre[:, chn * 512:(chn + 1) * 512],
                             start=True, stop=False, skip_group_check=True)
        for k in range(KW):
            for chn in range(NCH):
                nc.tensor.matmul(
                    hp[chn][:], vT_sb[:, k, :],
                    ut["h"][:, KW + k, chn * 512:(chn + 1) * 512],
                    start=False, stop=False, skip_group_check=True)
        rsT = small.tile([128, KW, Bc], bf16)
        transpose_to(rsT, rs_sb, alternate=True)
        for k in range(KW):
            for chn in range(NCH):
                nc.tensor.matmul(
                    hp[chn][:], rsT[:, k, :],
                    wt["h"][:, k, chn * 512:(chn + 1) * 512],
                    start=False, stop=(k == KW - 1), skip_group_check=True)

        # v in natural layout (needed only for the final +v)
        v_sb = tmp.tile([Bc, D], bf16, name="v_sb", tag="tmp3")
        for ch in range(KW):
            tpv = psT.tile([Bc, 128], bf16, name="tpv", tag="tp")
            nc.tensor.transpose(tpv[:], vT_sb[:, ch, :], ident[:, :])
            if ch % 2 == 1:
                nc.scalar.activation(out=v_sb[:, ch * 128:(ch + 1) * 128],
                                     in_=tpv[:], func=AF.Copy)
            else:
                nc.vector.tensor_copy(out=v_sb[:, ch * 128:(ch + 1) * 128],
                                      in_=tpv[:])
        # w = zs + v, in place, while the h matmuls run: the post-tanh chain
        # is then just (1-z)*h + w per chunk
        nc.vector.tensor_tensor(out=zs_sb[:], in0=zs_sb[:], in1=v_sb[:],
                                op=ALU.add)

        # combine per 512-chunk on DVE:
        #   s_t = (1-z)*h + zs + v,  (1-z)*h = (t_z*-0.5 + 0.5)*h
        ta = tmp.tile([Bc, D], f32, name="ta", tag="tmp")
        h_sb = tmp.tile([Bc, D], bf16, name="h_sb", tag="tmp2")
        for chn in range(NCH):
            c0, c1 = chn * 512, (chn + 1) * 512
            nc.scalar.activation(out=h_sb[:, c0:c1], in_=hp[chn][:],
                                 func=AF.Tanh)
            nc.vector.affine_mul_reduce(out=ta[:, c0:c1], accum_out=junk[:],
                                        in0=t_z[:, c0:c1], in1=h_sb[:, c0:c1],
                                        scale=-0.5, bias=0.5)
            nc.vector.tensor_tensor(out=ta[:, c0:c1], in0=ta[:, c0:c1],
                                    in1=zs_sb[:, c0:c1], op=ALU.add)
            nc.sync.dma_start(o_d[:, c0:c1], ta[:, c0:c1])
        es.close()

      if loop_n == 1:
          for rep in range(body_reps):
              body(rep)
      else:
          # weights must load outside the hardware loop (once, resident)
          load_weights()
          with tc.For_i(0, loop_n, 1) as i:
              body(i)
      perm_es.close()

    nc.compile()
    return nc


_NC_CACHE = {}


def _get_nc(loop_n=1):
    if loop_n not in _NC_CACHE:
        _NC_CACHE[loop_n] = _build(loop_n=loop_n)
    return _NC_CACHE[loop_n]


def _in_maps(inputs):
    import ml_dtypes
    bf = ml_dtypes.bfloat16
    x = np.asarray(inputs["x"], np.float32)
    st = np.asarray(inputs["state"], np.float32)
    cn = np.asarray(inputs["constants"], np.float32)
    x_b, st_b, cn_b = x.astype(bf), st.astype(bf), cn.astype(bf)
    shared = {"w_att": np.asarray(inputs["w_att"], np.float32).astype(bf)}
    for g in "zrh":
        for nm in (f"w_{g}", f"u_{g}", f"b_{g}"):
            shared[nm] = np.asarray(inputs[nm], np.float32).astype(bf)
    maps = []
    for c in range(N_CORES):
        lo, hi = c * Bc, (c + 1) * Bc
        m = dict(shared)
        m["x"] = np.ascontiguousarray(x_b[lo:hi])
        m["state"] = np.ascontiguousarray(st_b[lo:hi])
        m["constants"] = np.ascontiguousarray(cn_b[lo:hi])
        maps.append(m)
    return maps


def kernel(**inputs) -> np.ndarray:
    nc = _get_nc(loop_n=1)
    res = run_bass_kernel_spmd(nc, _in_maps(inputs),
                               core_ids=list(range(N_CORES)))
    return np.concatenate([res.results[c]["out"] for c in range(N_CORES)],
                          axis=0).astype(np.float32)
